# revision 3
# baseline (speedup 1.0000x reference)
"""Trainium2 Bass kernel for nn_CausalSelfAttention (B=4, L=2048, D=1024, H=16).

Sharding: 2 heads per core (tensor parallel) x 8 cores, all batches on every
core.  Each core computes qkv for its 2 heads over all tokens (reading full x),
runs causal attention, and produces a partial projection output
yT_c = proj_w[rows_c].T @ O_c^T  of shape [D, B*L].  The host sums the 8
partials, transposes, and adds proj_b.

Device pipeline per core (all matmuls in fp32r: 1 cycle/row at N>=256):
  x [tok,d] --PE transpose--> xT tiles [d,tok]
  qkv [tok,feat] = xT.T @ w  (+bias via K=1 matmul), RoPE on q,k in tok-major
  q,k --PE transpose--> QT [hd,tok] (transient), KT [hd,tok] (per-batch)
  S^T[k,q] = KT_slice.T @ QT_slice ; P^T = exp(S^T/8) (ACT, causal via
  affine_select on diagonal stripes) ; O^T[hd+1,q] += [V|1].T @ P^T
  O rows /= rowsum (reciprocal + gpsimd partition_broadcast + DVE mul)
  yT += pw_h.T @ OT_h  (two K=64 matmuls per out tile), PSUM->SBUF->DRAM
"""

import numpy as np

import concourse.bass as bass  # noqa: F401
import concourse.tile as tile
from concourse import mybir, bacc
from concourse import bass_utils
from concourse.masks import make_identity

f32 = mybir.dt.float32
f32r = mybir.dt.float32r
AL = mybir.AluOpType
AF = mybir.ActivationFunctionType

HIDDEN = 1024
HEADS = 16
HD = 64
ROPE_BASE = 10000.0
N_CORES = 8
H2 = 2           # heads per core
F = 3 * H2 * HD  # 384 qkv feature columns per core
TCH = 512        # qkv token chunk
QCH = 512        # attention q granule (= one PSUM bank wide)
DT = HIDDEN // 128  # 8 d tiles


def build_program(NB, T):
    """Build the per-core Bass program: NB batches of T tokens each."""
    assert T % TCH == 0
    NTOK = NB * T
    NKT = T // 128  # k tiles per batch
    nc = bacc.Bacc("TRN2", target_bir_lowering=False, debug=False,
                   num_devices=N_CORES)

    x = nc.dram_tensor("x", [NTOK, HIDDEN], f32r, kind="ExternalInput").ap()
    w = nc.dram_tensor("w", [HIDDEN, F], f32r, kind="ExternalInput").ap()
    brow = nc.dram_tensor("brow", [1, F], f32r, kind="ExternalInput").ap()
    pw = nc.dram_tensor("pw", [2, HD, HIDDEN], f32r, kind="ExternalInput").ap()
    cos_t = nc.dram_tensor("cos_t", [128, T], f32, kind="ExternalInput").ap()
    spm_t = nc.dram_tensor("spm_t", [128, T], f32, kind="ExternalInput").ap()
    yT = nc.dram_tensor("yT", [HIDDEN, NTOK], f32, kind="ExternalOutput").ap()

    with tile.TileContext(nc) as tc:
        with tc.tile_pool(name="const", bufs=1) as constp, \
             tc.tile_pool(name="resident", bufs=1) as resp, \
             tc.tile_pool(name="xload", bufs=6) as xp, \
             tc.tile_pool(name="xt", bufs=12) as xtp, \
             tc.tile_pool(name="rope", bufs=4) as ropep, \
             tc.tile_pool(name="qtcur", bufs=2) as qtp, \
             tc.tile_pool(name="pt", bufs=4) as ptp, \
             tc.tile_pool(name="ot", bufs=3) as otp, \
             tc.tile_pool(name="ysb", bufs=3) as yp, \
             tc.tile_pool(name="small", bufs=4) as smp, \
             tc.tile_pool(name="ps_s", bufs=2, space="PSUM") as ps_s_p, \
             tc.tile_pool(name="ps_o", bufs=2, space="PSUM") as ps_o_p, \
             tc.tile_pool(name="ps_m", bufs=2, space="PSUM") as ps_m_p, \
             tc.tile_pool(name="ps_qt", bufs=1, space="PSUM") as ps_qt_p, \
             tc.tile_pool(name="ps_kt", bufs=1, space="PSUM") as ps_kt_p:

            # ---- constants / residents ----
            ident_f = constp.tile([128, 128], f32)
            make_identity(nc, ident_f[:])
            ident = constp.tile([128, 128], f32r)
            nc.vector.tensor_copy(ident[:], ident_f[:])
            w_sb = constp.tile([128, DT * F], f32r)   # 8 tiles of [128, F]
            for dt in range(DT):
                nc.sync.dma_start(w_sb[:, dt * F:(dt + 1) * F],
                                  w[dt * 128:(dt + 1) * 128, :])
            brow_sb = constp.tile([1, F], f32r)
            nc.sync.dma_start(brow_sb[:], brow[:])
            ones_f = constp.tile([128, 128], f32)
            nc.gpsimd.memset(ones_f[:], 1.0)
            ones_row = constp.tile([1, 128], f32r)
            nc.vector.tensor_copy(ones_row[:], ones_f[0:1, :])
            pw_sb = constp.tile([64, 2 * HIDDEN], f32r)  # [64, 1024] per head
            for h in range(2):
                nc.sync.dma_start(pw_sb[:, h * HIDDEN:(h + 1) * HIDDEN], pw[h])
            cos_sb = constp.tile([128, T], f32)
            nc.sync.dma_start(cos_sb[:], cos_t[:])
            spm_sb = constp.tile([128, T], f32)
            nc.sync.dma_start(spm_sb[:], spm_t[:])

            KT_res = resp.tile([128, T], f32r)
            V_res = resp.tile([128, NKT * 130], f32r)
            v4 = V_res[:].rearrange("p (kt h c) -> p kt h c", kt=NKT, h=2)
            nc.gpsimd.tensor_copy(
                v4[:, :, :, 64],
                ones_f[:, :2 * NKT].rearrange("p (kt h) -> p kt h", kt=NKT))

            for b in range(NB):
                for qc in range(T // QCH):
                    Q0 = qc * QCH
                    t0 = b * T + Q0
                    QT_cur = qtp.tile([128, QCH], f32r, tag="qt")
                    # ---------- qkv + rope for tokens [t0, t0+QCH) ----------
                    xa = [xp.tile([128, HIDDEN], f32r, tag="x", name=f"xa{tt}")
                          for tt in range(4)]
                    for tt in range(4):
                        nc.sync.dma_start(
                            xa[tt][:], x[t0 + tt * 128: t0 + (tt + 1) * 128, :])
                    xt_sb = [xtp.tile([128, TCH], f32r, tag="xt", name=f"xt{dt}")
                             for dt in range(DT)]
                    for dt in range(DT):
                        ps_xt = ps_m_p.tile([128, TCH], f32r, tag="m")
                        for tt in range(4):
                            nc.tensor.transpose(
                                ps_xt[:, tt * 128:(tt + 1) * 128],
                                xa[tt][:, dt * 128:(dt + 1) * 128], ident[:])
                        if dt % 2 == 0:
                            nc.vector.tensor_copy(xt_sb[dt][:], ps_xt[:])
                        else:
                            nc.scalar.copy(xt_sb[dt][:], ps_xt[:])
                    ps_qt = ps_qt_p.tile([128, TCH], f32r, tag="qt")
                    ps_kt = ps_kt_p.tile([128, TCH], f32r, tag="kt")
                    for tt in range(4):
                        ps_q = ps_m_p.tile([128, F], f32, tag="m")
                        for dt in range(DT):
                            nc.tensor.matmul(
                                ps_q[:],
                                xt_sb[dt][:, tt * 128:(tt + 1) * 128],
                                w_sb[:, dt * F:(dt + 1) * F],
                                start=(dt == 0), stop=False)
                        nc.tensor.matmul(ps_q[:], ones_row[:], brow_sb[:],
                                         start=False, stop=True)
                        # rope in tok-major; pairs (j, j+32) inside each 64-blk
                        tcol = Q0 + tt * 128  # within-batch position base
                        cosv = cos_sb[:, tcol:tcol + 128]
                        spmv = spm_sb[:, tcol:tcol + 128]
                        s4 = spmv.rearrange("p (h s j) -> p h s j", h=2, s=2)
                        for qk in range(2):  # 0=q, 1=k
                            src = ps_q[:, qk * 128:(qk + 1) * 128]
                            sr4 = src.rearrange("p (h s j) -> p h s j",
                                                h=2, s=2)
                            t1 = ropep.tile([128, 128], f32, tag="t1")
                            nc.vector.tensor_tensor(t1[:], src, cosv, AL.mult)
                            t2 = ropep.tile([128, 128], f32, tag="t2")
                            t24 = t2[:].rearrange("p (h s j) -> p h s j",
                                                  h=2, s=2)
                            nc.vector.tensor_tensor(
                                t24[:, :, 0], sr4[:, :, 1], s4[:, :, 0], AL.mult)
                            nc.vector.tensor_tensor(
                                t24[:, :, 1], sr4[:, :, 0], s4[:, :, 1], AL.mult)
                            ro = ropep.tile([128, 128], f32r, tag="ro")
                            nc.vector.tensor_tensor(ro[:], t1[:], t2[:], AL.add)
                            nc.tensor.transpose(
                                (ps_qt if qk == 0 else ps_kt)
                                [:, tt * 128:(tt + 1) * 128], ro[:], ident[:])
                        # v: copy ps_q[:, 256:384] into V_res fancy cols
                        kt = Q0 // 128 + tt
                        vsrc = ps_q[:, 256:384].rearrange("p (h j) -> p h j",
                                                          h=2)
                        nc.vector.tensor_copy(v4[:, kt, :, 0:64], vsrc)
                    nc.scalar.copy(QT_cur[:], ps_qt[:])
                    nc.scalar.copy(KT_res[:, Q0:Q0 + QCH], ps_kt[:])
                    # ---------- attention for (b, qc) ----------
                    nkb = (Q0 + QCH) // 128
                    otstash = {}
                    for h in range(2):
                        hp = slice(64 * h, 64 * h + 64)
                        O = ps_o_p.tile([65, 512], f32, tag="o")
                        for kb in range(nkb):
                            qstart = max(0, 128 * kb - Q0)
                            ps_sc = ps_s_p.tile([128, QCH], f32, tag="s")
                            nc.tensor.matmul(
                                ps_sc[:, qstart:QCH],
                                KT_res[hp, kb * 128:(kb + 1) * 128],
                                QT_cur[hp, qstart:QCH],
                                start=True, stop=True)
                            pt = ptp.tile([128, QCH], f32r, tag="pt")
                            nc.scalar.activation(pt[:, qstart:QCH],
                                                 ps_sc[:, qstart:QCH],
                                                 AF.Exp, bias=0.0, scale=0.125)
                            if 128 * kb >= Q0:
                                ds = 128 * kb - Q0
                                nc.gpsimd.affine_select(
                                    out=pt[:, ds:ds + 128],
                                    in_=pt[:, ds:ds + 128],
                                    pattern=[[1, 128]], compare_op=AL.is_ge,
                                    fill=0.0, base=0, channel_multiplier=-1)
                            nc.tensor.matmul(
                                O[:, qstart:QCH],
                                V_res[:, 130 * kb + 65 * h:
                                      130 * kb + 65 * h + 65],
                                pt[:, qstart:QCH],
                                start=(kb == 0), stop=(kb == nkb - 1))
                        rs_sb = smp.tile([1, 512], f32, tag="rs")
                        nc.vector.reciprocal(rs_sb[:], O[64:65, :])
                        rsb = smp.tile([64, 512], f32, tag="rsb")
                        nc.gpsimd.partition_broadcast(rsb[:], rs_sb[:])
                        ot_t = otp.tile([64, 512], f32r, tag=f"ot{h}")
                        nc.vector.tensor_tensor(ot_t[:], O[0:64, :], rsb[:],
                                                AL.mult)
                        otstash[h] = ot_t
                    # ---------- projection for (b, qc) ----------
                    for ot in range(8):
                        ps_y = ps_m_p.tile([128, 512], f32, tag="m")
                        for h in range(2):
                            nc.tensor.matmul(
                                ps_y[:],
                                pw_sb[:, h * HIDDEN + ot * 128:
                                      h * HIDDEN + (ot + 1) * 128],
                                otstash[h][:],
                                start=(h == 0), stop=(h == 1))
                        ysb = yp.tile([128, 512], f32, tag="y")
                        if ot % 2 == 0:
                            nc.vector.tensor_copy(ysb[:], ps_y[:])
                        else:
                            nc.scalar.copy(ysb[:], ps_y[:])
                        nc.sync.dma_start(
                            yT[ot * 128:(ot + 1) * 128, t0:t0 + 512], ysb[:])
    nc.compile()
    return nc


# ---------------------------------------------------------------- host side

def _rope_tables(T):
    inv_freq = 1.0 / (ROPE_BASE ** (np.arange(0, HD, 2, dtype=np.float64) / HD))
    pos = np.arange(T, dtype=np.float64)
    ang = np.outer(pos, inv_freq)          # [T, 32]
    cos = np.cos(ang).astype(np.float32)
    sin = np.sin(ang).astype(np.float32)
    j = np.arange(128)
    jm32 = j % 32
    x1mask = (j % 64) < 32
    cos_t = np.empty((128, T), np.float32)
    spm_t = np.empty((128, T), np.float32)
    for tt in range(T // 128):
        blk_c = cos[tt * 128:(tt + 1) * 128][:, jm32]          # [128,128]
        blk_s = sin[tt * 128:(tt + 1) * 128][:, jm32]
        blk_s = np.where(x1mask[None, :], -blk_s, blk_s)
        cos_t[:, tt * 128:(tt + 1) * 128] = blk_c
        spm_t[:, tt * 128:(tt + 1) * 128] = blk_s
    return cos_t, spm_t


def make_core_inputs(x, qkv_w, qkv_b, proj_w, NB, T):
    NTOK = NB * T
    xf = np.ascontiguousarray(
        np.asarray(x).reshape(NTOK, HIDDEN).astype(np.float32))
    cos_t, spm_t = _rope_tables(T)
    in_maps = []
    for c in range(N_CORES):
        col = HD * H2 * c
        wq = qkv_w[:, col:col + 128]
        wk = qkv_w[:, HIDDEN + col:HIDDEN + col + 128]
        wv = qkv_w[:, 2 * HIDDEN + col:2 * HIDDEN + col + 128]
        wc = np.ascontiguousarray(
            np.concatenate([wq, wk, wv], axis=1).astype(np.float32))
        bq = qkv_b[col:col + 128]
        bk = qkv_b[HIDDEN + col:HIDDEN + col + 128]
        bv = qkv_b[2 * HIDDEN + col:2 * HIDDEN + col + 128]
        browc = np.ascontiguousarray(
            np.concatenate([bq, bk, bv])[None, :].astype(np.float32))
        pwc = np.ascontiguousarray(
            proj_w[col:col + 128, :].reshape(2, HD, HIDDEN).astype(np.float32))
        in_maps.append({
            "x": xf, "w": wc, "brow": browc, "pw": pwc,
            "cos_t": cos_t, "spm_t": spm_t,
        })
    return in_maps


_PROGRAM_CACHE = {}


def _get_program(NB, T):
    key = (NB, T)
    if key not in _PROGRAM_CACHE:
        _PROGRAM_CACHE[key] = build_program(NB, T)
    return _PROGRAM_CACHE[key]


def run(x, qkv_w, qkv_b, proj_w, proj_b, NB, T, trace=False):
    nc = _get_program(NB, T)
    in_maps = make_core_inputs(x, qkv_w, qkv_b, proj_w, NB, T)
    res = bass_utils.run_bass_kernel_spmd(
        nc, in_maps, core_ids=list(range(N_CORES)), trace=trace)
    acc = res.results[0]["yT"].astype(np.float32).copy()
    for c in range(1, N_CORES):
        acc += res.results[c]["yT"]
    out = (acc.T.reshape(NB, T, HIDDEN)
           + np.asarray(proj_b)[None, None, :].astype(np.float32))
    return out, res


def kernel(x, qkv_w, qkv_b, proj_w, proj_b):
    x = np.asarray(x)
    B, L, D = x.shape
    out, _ = run(x, np.asarray(qkv_w), np.asarray(qkv_b),
                 np.asarray(proj_w), np.asarray(proj_b), NB=B, T=L)
    return out.astype(np.float32)


# revision 9
# speedup vs baseline: 1.3579x; 1.3579x over previous
"""Trainium2 Bass kernel for nn_CausalSelfAttention (B=4, L=2048, D=1024, H=16).

Sharding: 2 heads per core (tensor parallel) x 8 cores, all batches on every
core.  Each core computes qkv for its 2 heads over all tokens (reading full x),
runs causal attention, and produces a partial projection output
yT_c = proj_w[rows_c].T @ O_c^T  of shape [D, B*L].  The host sums the 8
partials, transposes, and adds proj_b.

Device pipeline per core (all matmuls fp32r: 1 cycle/row at N>=256):
  x [tok,d] --PE transpose--> xT tiles [d,tok]
  qkvT [feat,tok] = w_tile.T @ xT (+bias via K=1 matmul)
  RoPE: rot(q) via signed-permutation matmul on PE, then
        qT_roped = qT*cos + rot(qT)*sin  (3 full-height DVE ops)
  V: PE-transpose back to tok-major, stored as [V|1] tiles
  S^T[k,q] = KT_slice.T @ QT_slice (heads interleaved -> PE row-group overlap)
  P^T = exp(S^T/8) (ACT, causal via affine_select on diagonal stripes)
  O^T[hd+1,q] += [V|1].T @ P^T
  normalize via reciprocal_approx_fast + gpsimd partition_broadcast + DVE mul
  yT += pw_h.T @ OT_h (two K=64 matmuls per tile)
The normalize+projection of iteration i-1 is emitted after the qkv phase of
iteration i so the PE never idles on the normalization chain (HAM stays warm).
"""

import numpy as np

import concourse.bass as bass  # noqa: F401
import concourse.tile as tile
from concourse import mybir, bacc
from concourse import bass_utils
from concourse.masks import make_identity

f32 = mybir.dt.float32
f32r = mybir.dt.float32r
AL = mybir.AluOpType
AF = mybir.ActivationFunctionType

HIDDEN = 1024
HEADS = 16
HD = 64
ROPE_BASE = 10000.0
N_CORES = 8
H2 = 2           # heads per core
F = 3 * H2 * HD  # 384 qkv feature columns per core
QCH = 512        # token chunk = attention q granule
DT = HIDDEN // 128  # 8 d tiles


def build_program(NB, T):
    """Build the per-core Bass program: NB batches of T tokens each."""
    assert T % QCH == 0
    NTOK = NB * T
    NKT = T // 128  # k tiles per batch
    nc = bacc.Bacc("TRN2", target_bir_lowering=False, debug=False,
                   num_devices=N_CORES)

    x = nc.dram_tensor("x", [NTOK, HIDDEN], f32r, kind="ExternalInput").ap()
    w = nc.dram_tensor("w", [HIDDEN, F], f32r, kind="ExternalInput").ap()
    brow = nc.dram_tensor("brow", [1, F], f32r, kind="ExternalInput").ap()
    psgn = nc.dram_tensor("psgn", [128, 128], f32r, kind="ExternalInput").ap()
    pw = nc.dram_tensor("pw", [2, HD, HIDDEN], f32r, kind="ExternalInput").ap()
    cos_t = nc.dram_tensor("cos_t", [128, T], f32, kind="ExternalInput").ap()
    sin_t = nc.dram_tensor("sin_t", [128, T], f32, kind="ExternalInput").ap()
    yT = nc.dram_tensor("yT", [HIDDEN, NTOK], f32, kind="ExternalOutput").ap()

    with tile.TileContext(nc) as tc:
        with tc.tile_pool(name="const", bufs=1) as constp, \
             tc.tile_pool(name="resident", bufs=1) as resp, \
             tc.tile_pool(name="xload", bufs=6) as xp, \
             tc.tile_pool(name="xt", bufs=12) as xtp, \
             tc.tile_pool(name="rope", bufs=3) as ropep, \
             tc.tile_pool(name="qtcur", bufs=2) as qtp, \
             tc.tile_pool(name="pt", bufs=4) as ptp, \
             tc.tile_pool(name="ot", bufs=3) as otp, \
             tc.tile_pool(name="ysb", bufs=3) as yp, \
             tc.tile_pool(name="small", bufs=4) as smp, \
             tc.tile_pool(name="ps_s", bufs=2, space="PSUM") as ps_s_p, \
             tc.tile_pool(name="ps_o", bufs=4, space="PSUM") as ps_o_p, \
             tc.tile_pool(name="ps_m", bufs=2, space="PSUM") as ps_m_p:

            # ---- constants / residents ----
            ident_f = constp.tile([128, 128], f32)
            make_identity(nc, ident_f[:])
            ident = constp.tile([128, 128], f32r)
            nc.vector.tensor_copy(ident[:], ident_f[:])
            # w tiles: per d-tile, F columns
            w_sb = constp.tile([128, DT * F], f32r)
            for dt in range(DT):
                nc.sync.dma_start(w_sb[:, dt * F:(dt + 1) * F],
                                  w[dt * 128:(dt + 1) * 128, :])
            brow_sb = constp.tile([1, F], f32r)
            nc.sync.dma_start(brow_sb[:], brow[:])
            psgn_sb = constp.tile([128, 128], f32r)
            nc.sync.dma_start(psgn_sb[:], psgn[:])
            ones_f = constp.tile([128, 512], f32)
            nc.gpsimd.memset(ones_f[:], 1.0)
            ones_row = constp.tile([1, 512], f32r)
            nc.vector.tensor_copy(ones_row[:], ones_f[0:1, :])
            pw_sb = constp.tile([64, 2 * HIDDEN], f32r)
            for h in range(2):
                nc.sync.dma_start(pw_sb[:, h * HIDDEN:(h + 1) * HIDDEN], pw[h])
            cos_sb = constp.tile([128, T], f32)
            nc.sync.dma_start(cos_sb[:], cos_t[:])
            sin_sb = constp.tile([128, T], f32)
            nc.sync.dma_start(sin_sb[:], sin_t[:])

            KT_res = resp.tile([128, T], f32r)
            V_res = resp.tile([128, NKT * 130], f32r)
            v4 = V_res[:].rearrange("p (kt h c) -> p kt h c", kt=NKT, h=2)
            nc.gpsimd.tensor_copy(
                v4[:, :, :, 64],
                ones_f[:, :2 * NKT].rearrange("p (kt h) -> p kt h", kt=NKT))

            def norm_proj(st):
                O, t0v = st
                ots = []
                for h in range(2):
                    rs_sb = smp.tile([1, 512], f32, tag="rs", name="rs")
                    nc.vector.reciprocal(rs_sb[:], O[h][64:65, :])
                    rsb = smp.tile([64, 512], f32, tag="rsb", name="rsb")
                    nc.gpsimd.partition_broadcast(rsb[:], rs_sb[:])
                    ot_t = otp.tile([64, 512], f32r, tag=f"ot{h}",
                                    name=f"ot{h}")
                    nc.vector.tensor_tensor(ot_t[:], O[h][0:64, :], rsb[:],
                                            AL.mult)
                    ots.append(ot_t)
                for oi in range(8):
                    ps_y = ps_m_p.tile([128, 512], f32, tag="m", name="ps_y")
                    for h in range(2):
                        nc.tensor.matmul(
                            ps_y[:],
                            pw_sb[:, h * HIDDEN + oi * 128:
                                  h * HIDDEN + (oi + 1) * 128],
                            ots[h][:], start=(h == 0), stop=(h == 1))
                    ysb = yp.tile([128, 512], f32, tag="y", name="ysb")
                    if oi % 2 == 0:
                        nc.vector.tensor_copy(ysb[:], ps_y[:])
                    else:
                        nc.scalar.copy(ysb[:], ps_y[:])
                    nc.sync.dma_start(
                        yT[oi * 128:(oi + 1) * 128, t0v:t0v + 512], ysb[:])

            prev = None
            for b in range(NB):
                for qc in range(T // QCH):
                    Q0 = qc * QCH
                    t0 = b * T + Q0
                    # ---------- qkv phase for tokens [t0, t0+512) ----------
                    xa = [xp.tile([128, HIDDEN], f32r, tag="x", name=f"xa{tt}")
                          for tt in range(4)]
                    for tt in range(4):
                        nc.sync.dma_start(
                            xa[tt][:], x[t0 + tt * 128: t0 + (tt + 1) * 128, :])
                    xt_sb = [xtp.tile([128, QCH], f32r, tag="xt",
                                      name=f"xt{dt}") for dt in range(DT)]
                    for dt in range(DT):
                        ps_xt = ps_m_p.tile([128, QCH], f32r, tag="m",
                                            name="ps_xt")
                        for tt in range(4):
                            nc.tensor.transpose(
                                ps_xt[:, tt * 128:(tt + 1) * 128],
                                xa[tt][:, dt * 128:(dt + 1) * 128], ident[:])
                        nc.vector.tensor_copy(xt_sb[dt][:], ps_xt[:])
                    QT_cur = qtp.tile([128, QCH], f32r, tag="qt", name="QT")
                    for f in range(3):  # 0=q, 1=k, 2=v
                        ps_f = ps_m_p.tile([128, QCH], f32, tag="m",
                                           name="ps_f")
                        for dt in range(DT):
                            nc.tensor.matmul(
                                ps_f[:],
                                w_sb[:, dt * F + f * 128:dt * F + (f + 1) * 128],
                                xt_sb[dt][:], start=(dt == 0), stop=False)
                        nc.tensor.matmul(
                            ps_f[:], brow_sb[:, f * 128:(f + 1) * 128],
                            ones_row[:], start=False, stop=True)
                        raw = ropep.tile([128, QCH], f32r, tag="raw",
                                         name="raw")
                        nc.scalar.copy(raw[:], ps_f[:])
                        if f < 2:
                            ps_rot = ps_m_p.tile([128, QCH], f32, tag="m",
                                                 name="ps_rot")
                            nc.tensor.matmul(ps_rot[:], psgn_sb[:], raw[:],
                                             start=True, stop=True)
                            t1 = ropep.tile([128, QCH], f32, tag="t1",
                                            name="t1")
                            nc.vector.tensor_tensor(
                                t1[:], raw[:], cos_sb[:, Q0:Q0 + QCH], AL.mult)
                            t2 = ropep.tile([128, QCH], f32, tag="t2",
                                            name="t2")
                            nc.vector.tensor_tensor(
                                t2[:], ps_rot[:], sin_sb[:, Q0:Q0 + QCH],
                                AL.mult)
                            dst = (QT_cur[:] if f == 0
                                   else KT_res[:, Q0:Q0 + QCH])
                            nc.vector.tensor_tensor(dst, t1[:], t2[:], AL.add)
                        else:
                            for tt in range(4):
                                ps_vt = ps_m_p.tile([128, 128], f32r, tag="m",
                                                    name="ps_vt")
                                nc.tensor.transpose(
                                    ps_vt[:],
                                    raw[:, tt * 128:(tt + 1) * 128], ident[:])
                                kt = Q0 // 128 + tt
                                nc.vector.tensor_copy(
                                    v4[:, kt, :, 0:64],
                                    ps_vt[:].rearrange("p (h j) -> p h j", h=2))
                    # ---------- deferred normalize+projection ----------
                    if prev is not None:
                        norm_proj(prev)
                    # ---------- attention for (b, qc) ----------
                    nkb = (Q0 + QCH) // 128
                    O = [ps_o_p.tile([65, 512], f32, tag="o", name=f"O{h}")
                         for h in range(2)]
                    for kb in range(nkb):
                        qstart = max(0, 128 * kb - Q0)
                        for h in range(2):
                            hp = slice(64 * h, 64 * h + 64)
                            ps_sc = ps_s_p.tile([128, QCH], f32, tag="s",
                                                name="ps_sc")
                            nc.tensor.matmul(
                                ps_sc[:, qstart:QCH],
                                KT_res[hp, kb * 128:(kb + 1) * 128],
                                QT_cur[hp, qstart:QCH],
                                start=True, stop=True)
                            pt = ptp.tile([128, QCH], f32r, tag="pt",
                                          name="pt")
                            nc.scalar.activation(pt[:, qstart:QCH],
                                                 ps_sc[:, qstart:QCH],
                                                 AF.Exp, bias=0.0, scale=0.125)
                            if 128 * kb >= Q0:
                                ds = 128 * kb - Q0
                                nc.gpsimd.affine_select(
                                    out=pt[:, ds:ds + 128],
                                    in_=pt[:, ds:ds + 128],
                                    pattern=[[1, 128]], compare_op=AL.is_ge,
                                    fill=0.0, base=0, channel_multiplier=-1)
                            nc.tensor.matmul(
                                O[h][:, qstart:QCH],
                                V_res[:, 130 * kb + 65 * h:
                                      130 * kb + 65 * h + 65],
                                pt[:, qstart:QCH],
                                start=(kb == 0), stop=(kb == nkb - 1))
                    prev = (O, t0)
            norm_proj(prev)
    nc.compile()
    return nc


# ---------------------------------------------------------------- host side

def _rope_tables(T):
    inv_freq = 1.0 / (ROPE_BASE ** (np.arange(0, HD, 2, dtype=np.float64) / HD))
    pos = np.arange(T, dtype=np.float64)
    ang = np.outer(pos, inv_freq)          # [T, 32]
    cos = np.cos(ang).astype(np.float32)   # [T, 32]
    sin = np.sin(ang).astype(np.float32)
    jm32 = np.arange(128) % 32
    # feat-major: row r (feature), col t (within-batch position)
    cos_t = np.ascontiguousarray(cos[:, jm32].T)   # [128, T]
    sin_t = np.ascontiguousarray(sin[:, jm32].T)
    return cos_t, sin_t


def _psgn():
    p = np.zeros((HD, HD), np.float32)
    for i in range(32):
        p[i + 32, i] = -1.0   # out dim i (<32) = -in dim i+32
        p[i, i + 32] = 1.0    # out dim i+32   = +in dim i
    pf = np.zeros((128, 128), np.float32)
    pf[0:64, 0:64] = p        # head A block
    pf[64:128, 64:128] = p    # head B block
    return np.ascontiguousarray(pf)


def make_core_inputs(x, qkv_w, qkv_b, proj_w, NB, T):
    NTOK = NB * T
    xf = np.ascontiguousarray(
        np.asarray(x).reshape(NTOK, HIDDEN).astype(np.float32))
    cos_t, sin_t = _rope_tables(T)
    psgn = _psgn()
    in_maps = []
    for c in range(N_CORES):
        col = HD * H2 * c
        wq = qkv_w[:, col:col + 128]
        wk = qkv_w[:, HIDDEN + col:HIDDEN + col + 128]
        wv = qkv_w[:, 2 * HIDDEN + col:2 * HIDDEN + col + 128]
        wc = np.ascontiguousarray(
            np.concatenate([wq, wk, wv], axis=1).astype(np.float32))
        bq = qkv_b[col:col + 128]
        bk = qkv_b[HIDDEN + col:HIDDEN + col + 128]
        bv = qkv_b[2 * HIDDEN + col:2 * HIDDEN + col + 128]
        browc = np.ascontiguousarray(
            np.concatenate([bq, bk, bv])[None, :].astype(np.float32))
        pwc = np.ascontiguousarray(
            proj_w[col:col + 128, :].reshape(2, HD, HIDDEN).astype(np.float32))
        in_maps.append({
            "x": xf, "w": wc, "brow": browc, "psgn": psgn, "pw": pwc,
            "cos_t": cos_t, "sin_t": sin_t,
        })
    return in_maps


_PROGRAM_CACHE = {}


def _get_program(NB, T):
    key = (NB, T)
    if key not in _PROGRAM_CACHE:
        _PROGRAM_CACHE[key] = build_program(NB, T)
    return _PROGRAM_CACHE[key]


def run(x, qkv_w, qkv_b, proj_w, proj_b, NB, T, trace=False):
    nc = _get_program(NB, T)
    in_maps = make_core_inputs(x, qkv_w, qkv_b, proj_w, NB, T)
    res = bass_utils.run_bass_kernel_spmd(
        nc, in_maps, core_ids=list(range(N_CORES)), trace=trace)
    acc = res.results[0]["yT"].astype(np.float32).copy()
    for c in range(1, N_CORES):
        acc += res.results[c]["yT"]
    out = (acc.T.reshape(NB, T, HIDDEN)
           + np.asarray(proj_b)[None, None, :].astype(np.float32))
    return out, res


def kernel(x, qkv_w, qkv_b, proj_w, proj_b):
    x = np.asarray(x)
    B, L, D = x.shape
    out, _ = run(x, np.asarray(qkv_w), np.asarray(qkv_b),
                 np.asarray(proj_w), np.asarray(proj_b), NB=B, T=L)
    return out.astype(np.float32)


# revision 13
# speedup vs baseline: 1.4843x; 1.0931x over previous
"""Trainium2 Bass kernel for nn_CausalSelfAttention (B=4, L=2048, D=1024, H=16).

Sharding: 2 heads per core (tensor parallel) x 8 cores, all batches on every
core.  Each core computes qkv for its 2 heads over all tokens (reading full x),
runs causal attention, and produces a partial projection output
yT_c = proj_w[rows_c].T @ O_c^T  of shape [D, B*L].  The host sums the 8
partials, transposes, and adds proj_b.

Device pipeline per core (all matmuls fp32r: 1 cycle/row at N>=256):
  x [tok,d] --PE transpose--> xT tiles [d,tok]
  qkvT [feat,tok] = w_tile.T @ xT (+bias via K=1 matmul)
  RoPE: rot(q) via signed-permutation matmul on PE, then
        qT_roped = qT*cos + rot(qT)*sin  (3 full-height DVE ops)
  V: PE-transpose back to tok-major, stored as [V|1] tiles
  S^T[k,q] = KT_slice.T @ QT_slice (heads interleaved -> PE row-group overlap)
  P^T = exp(S^T/8) (ACT, causal via affine_select on diagonal stripes)
  O^T[hd+1,q] += [V|1].T @ P^T
  normalize via reciprocal_approx_fast + gpsimd partition_broadcast + DVE mul
  yT += pw_h.T @ OT_h (two K=64 matmuls per tile)
The normalize+projection of iteration i-1 is emitted after the qkv phase of
iteration i so the PE never idles on the normalization chain (HAM stays warm).
"""

import numpy as np

import concourse.bass as bass  # noqa: F401
import concourse.tile as tile
from concourse import mybir, bacc
from concourse import bass_utils
from concourse.masks import make_identity

f32 = mybir.dt.float32
f32r = mybir.dt.float32r
AL = mybir.AluOpType
AF = mybir.ActivationFunctionType

HIDDEN = 1024
HEADS = 16
HD = 64
ROPE_BASE = 10000.0
N_CORES = 8
H2 = 2           # heads per core
F = 3 * H2 * HD  # 384 qkv feature columns per core
QCH = 512        # token chunk = attention q granule
DT = HIDDEN // 128  # 8 d tiles


def build_program(NB, T):
    """Build the per-core Bass program: NB batches of T tokens each."""
    assert T % QCH == 0
    NTOK = NB * T
    NKT = T // 128  # k tiles per batch
    nc = bacc.Bacc("TRN2", target_bir_lowering=False, debug=False,
                   num_devices=N_CORES)

    x = nc.dram_tensor("x", [NTOK, HIDDEN], f32r, kind="ExternalInput").ap()
    w = nc.dram_tensor("w", [HIDDEN, F], f32r, kind="ExternalInput").ap()
    brow = nc.dram_tensor("brow", [1, F], f32r, kind="ExternalInput").ap()
    psgn = nc.dram_tensor("psgn", [128, 128], f32r, kind="ExternalInput").ap()
    pw = nc.dram_tensor("pw", [2, HD, HIDDEN], f32r, kind="ExternalInput").ap()
    cos_t = nc.dram_tensor("cos_t", [128, T], f32, kind="ExternalInput").ap()
    sin_t = nc.dram_tensor("sin_t", [128, T], f32, kind="ExternalInput").ap()
    yT = nc.dram_tensor("yT", [HIDDEN, NTOK], f32, kind="ExternalOutput").ap()

    with tile.TileContext(nc) as tc:
        with tc.tile_pool(name="const", bufs=1) as constp, \
             tc.tile_pool(name="resident", bufs=1) as resp, \
             tc.tile_pool(name="xload", bufs=6) as xp, \
             tc.tile_pool(name="xt", bufs=12) as xtp, \
             tc.tile_pool(name="rope", bufs=3) as ropep, \
             tc.tile_pool(name="qtcur", bufs=2) as qtp, \
             tc.tile_pool(name="pt", bufs=4) as ptp, \
             tc.tile_pool(name="ot", bufs=3) as otp, \
             tc.tile_pool(name="ysb", bufs=3) as yp, \
             tc.tile_pool(name="small", bufs=4) as smp, \
             tc.tile_pool(name="ps_s", bufs=2, space="PSUM") as ps_s_p, \
             tc.tile_pool(name="ps_o", bufs=4, space="PSUM") as ps_o_p, \
             tc.tile_pool(name="ps_m", bufs=2, space="PSUM") as ps_m_p:

            # ---- constants / residents ----
            ident_f = constp.tile([128, 128], f32)
            make_identity(nc, ident_f[:])
            ident = constp.tile([128, 128], f32r)
            nc.vector.tensor_copy(ident[:], ident_f[:])
            # w tiles: per d-tile, F columns
            w_sb = constp.tile([128, DT * F], f32r)
            for dt in range(DT):
                nc.sync.dma_start(w_sb[:, dt * F:(dt + 1) * F],
                                  w[dt * 128:(dt + 1) * 128, :])
            brow_sb = constp.tile([1, F], f32r)
            nc.sync.dma_start(brow_sb[:], brow[:])
            psgn_sb = constp.tile([128, 128], f32r)
            nc.sync.dma_start(psgn_sb[:], psgn[:])
            ones_f = constp.tile([128, 512], f32)
            nc.gpsimd.memset(ones_f[:], 1.0)
            ones_row = constp.tile([1, 512], f32r)
            nc.vector.tensor_copy(ones_row[:], ones_f[0:1, :])
            pw_sb = constp.tile([64, 2 * HIDDEN], f32r)
            for h in range(2):
                nc.sync.dma_start(pw_sb[:, h * HIDDEN:(h + 1) * HIDDEN], pw[h])
            cos_sb = constp.tile([128, T], f32)
            nc.sync.dma_start(cos_sb[:], cos_t[:])
            sin_sb = constp.tile([128, T], f32)
            nc.sync.dma_start(sin_sb[:], sin_t[:])

            KT_res = resp.tile([128, T], f32r)
            V_res = resp.tile([128, NKT * 130], f32r)
            v4 = V_res[:].rearrange("p (kt h c) -> p kt h c", kt=NKT, h=2)
            nc.gpsimd.tensor_copy(
                v4[:, :, :, 64],
                ones_f[:, :2 * NKT].rearrange("p (kt h) -> p kt h", kt=NKT))

            def norm_part(st):
                O, t0v = st
                ots = []
                for h in range(2):
                    # 1/rowsum = exp(-ln(rowsum)) on ACT (same table set as
                    # the attention Exp; DVE reciprocal is 3.3us and would
                    # stall the pipeline)
                    lnv = smp.tile([1, 512], f32, tag="ln", name="lnv")
                    nc.scalar.activation(lnv[:], O[h][64:65, :], AF.Ln)
                    rs_sb = smp.tile([1, 512], f32, tag="rs", name="rs")
                    nc.scalar.activation(rs_sb[:], lnv[:], AF.Exp,
                                         bias=0.0, scale=-1.0)
                    rsb = smp.tile([64, 512], f32, tag="rsb", name="rsb")
                    nc.gpsimd.partition_broadcast(rsb[:], rs_sb[:])
                    ot_t = otp.tile([64, 512], f32r, tag=f"ot{h}",
                                    name=f"ot{h}")
                    nc.vector.tensor_tensor(ot_t[:], O[h][0:64, :], rsb[:],
                                            AL.mult)
                    ots.append(ot_t)
                return ots

            def proj_part(st, ots):
                O, t0v = st
                for oi in range(8):
                    ps_y = ps_m_p.tile([128, 512], f32, tag="m", name="ps_y")
                    for h in range(2):
                        nc.tensor.matmul(
                            ps_y[:],
                            pw_sb[:, h * HIDDEN + oi * 128:
                                  h * HIDDEN + (oi + 1) * 128],
                            ots[h][:], start=(h == 0), stop=(h == 1))
                    ysb = yp.tile([128, 512], f32, tag="y", name="ysb")
                    if oi % 2 == 0:
                        nc.vector.tensor_copy(ysb[:], ps_y[:])
                    else:
                        nc.scalar.copy(ysb[:], ps_y[:])
                    nc.sync.dma_start(
                        yT[oi * 128:(oi + 1) * 128, t0v:t0v + 512], ysb[:])

            prev = None
            for b in range(NB):
                for qc in range(T // QCH):
                    Q0 = qc * QCH
                    t0 = b * T + Q0
                    prev_ots = norm_part(prev) if prev is not None else None
                    # ---------- qkv phase for tokens [t0, t0+512) ----------
                    xa = [xp.tile([128, HIDDEN], f32r, tag="x", name=f"xa{tt}")
                          for tt in range(4)]
                    for tt in range(4):
                        nc.sync.dma_start(
                            xa[tt][:], x[t0 + tt * 128: t0 + (tt + 1) * 128, :])
                    xt_sb = [xtp.tile([128, QCH], f32r, tag="xt",
                                      name=f"xt{dt}") for dt in range(DT)]
                    for dt in range(DT):
                        ps_xt = ps_m_p.tile([128, QCH], f32r, tag="m",
                                            name="ps_xt")
                        for tt in range(4):
                            nc.tensor.transpose(
                                ps_xt[:, tt * 128:(tt + 1) * 128],
                                xa[tt][:, dt * 128:(dt + 1) * 128], ident[:])
                        nc.vector.tensor_copy(xt_sb[dt][:], ps_xt[:])
                    QT_cur = qtp.tile([128, QCH], f32r, tag="qt", name="QT")
                    for f in range(3):  # 0=q, 1=k, 2=v
                        ps_f = ps_m_p.tile([128, QCH], f32, tag="m",
                                           name="ps_f")
                        for dt in range(DT):
                            nc.tensor.matmul(
                                ps_f[:],
                                w_sb[:, dt * F + f * 128:dt * F + (f + 1) * 128],
                                xt_sb[dt][:], start=(dt == 0), stop=False)
                        nc.tensor.matmul(
                            ps_f[:], brow_sb[:, f * 128:(f + 1) * 128],
                            ones_row[:], start=False, stop=True)
                        raw = ropep.tile([128, QCH], f32r, tag="raw",
                                         name="raw")
                        nc.scalar.copy(raw[:], ps_f[:])
                        if f < 2:
                            ps_rot = ps_m_p.tile([128, QCH], f32, tag="m",
                                                 name="ps_rot")
                            nc.tensor.matmul(ps_rot[:], psgn_sb[:], raw[:],
                                             start=True, stop=True)
                            t1 = ropep.tile([128, QCH], f32, tag="t1",
                                            name="t1")
                            nc.vector.tensor_tensor(
                                t1[:], raw[:], cos_sb[:, Q0:Q0 + QCH], AL.mult)
                            t2 = ropep.tile([128, QCH], f32, tag="t2",
                                            name="t2")
                            nc.vector.tensor_tensor(
                                t2[:], ps_rot[:], sin_sb[:, Q0:Q0 + QCH],
                                AL.mult)
                            dst = (QT_cur[:] if f == 0
                                   else KT_res[:, Q0:Q0 + QCH])
                            nc.vector.tensor_tensor(dst, t1[:], t2[:], AL.add)
                        else:
                            for tt in range(4):
                                ps_vt = ps_m_p.tile([128, 128], f32r, tag="m",
                                                    name="ps_vt")
                                nc.tensor.transpose(
                                    ps_vt[:],
                                    raw[:, tt * 128:(tt + 1) * 128], ident[:])
                                kt = Q0 // 128 + tt
                                nc.vector.tensor_copy(
                                    v4[:, kt, :, 0:64],
                                    ps_vt[:].rearrange("p (h j) -> p h j", h=2))
                    # ---------- deferred projection ----------
                    if prev is not None:
                        proj_part(prev, prev_ots)
                    # ---------- attention for (b, qc) ----------
                    nkb = (Q0 + QCH) // 128
                    O = [ps_o_p.tile([65, 512], f32, tag="o", name=f"O{h}")
                         for h in range(2)]
                    for kb in range(nkb):
                        qstart = max(0, 128 * kb - Q0)
                        for h in range(2):
                            hp = slice(64 * h, 64 * h + 64)
                            ps_sc = ps_s_p.tile([128, QCH], f32, tag="s",
                                                name="ps_sc")
                            nc.tensor.matmul(
                                ps_sc[:, qstart:QCH],
                                KT_res[hp, kb * 128:(kb + 1) * 128],
                                QT_cur[hp, qstart:QCH],
                                start=True, stop=True)
                            pt = ptp.tile([128, QCH], f32r, tag="pt",
                                          name="pt")
                            nc.scalar.activation(pt[:, qstart:QCH],
                                                 ps_sc[:, qstart:QCH],
                                                 AF.Exp, bias=0.0, scale=0.125)
                            if 128 * kb >= Q0:
                                ds = 128 * kb - Q0
                                nc.gpsimd.affine_select(
                                    out=pt[:, ds:ds + 128],
                                    in_=pt[:, ds:ds + 128],
                                    pattern=[[1, 128]], compare_op=AL.is_ge,
                                    fill=0.0, base=0, channel_multiplier=-1)
                            nc.tensor.matmul(
                                O[h][:, qstart:QCH],
                                V_res[:, 130 * kb + 65 * h:
                                      130 * kb + 65 * h + 65],
                                pt[:, qstart:QCH],
                                start=(kb == 0), stop=(kb == nkb - 1))
                    prev = (O, t0)
            proj_part(prev, norm_part(prev))
    nc.compile()
    return nc


# ---------------------------------------------------------------- host side

def _rope_tables(T):
    inv_freq = 1.0 / (ROPE_BASE ** (np.arange(0, HD, 2, dtype=np.float64) / HD))
    pos = np.arange(T, dtype=np.float64)
    ang = np.outer(pos, inv_freq)          # [T, 32]
    cos = np.cos(ang).astype(np.float32)   # [T, 32]
    sin = np.sin(ang).astype(np.float32)
    jm32 = np.arange(128) % 32
    # feat-major: row r (feature), col t (within-batch position)
    cos_t = np.ascontiguousarray(cos[:, jm32].T)   # [128, T]
    sin_t = np.ascontiguousarray(sin[:, jm32].T)
    return cos_t, sin_t


def _psgn():
    p = np.zeros((HD, HD), np.float32)
    for i in range(32):
        p[i + 32, i] = -1.0   # out dim i (<32) = -in dim i+32
        p[i, i + 32] = 1.0    # out dim i+32   = +in dim i
    pf = np.zeros((128, 128), np.float32)
    pf[0:64, 0:64] = p        # head A block
    pf[64:128, 64:128] = p    # head B block
    return np.ascontiguousarray(pf)


def make_core_inputs(x, qkv_w, qkv_b, proj_w, NB, T):
    NTOK = NB * T
    xf = np.ascontiguousarray(
        np.asarray(x).reshape(NTOK, HIDDEN).astype(np.float32))
    cos_t, sin_t = _rope_tables(T)
    psgn = _psgn()
    in_maps = []
    for c in range(N_CORES):
        col = HD * H2 * c
        wq = qkv_w[:, col:col + 128]
        wk = qkv_w[:, HIDDEN + col:HIDDEN + col + 128]
        wv = qkv_w[:, 2 * HIDDEN + col:2 * HIDDEN + col + 128]
        wc = np.ascontiguousarray(
            np.concatenate([wq, wk, wv], axis=1).astype(np.float32))
        bq = qkv_b[col:col + 128]
        bk = qkv_b[HIDDEN + col:HIDDEN + col + 128]
        bv = qkv_b[2 * HIDDEN + col:2 * HIDDEN + col + 128]
        browc = np.ascontiguousarray(
            np.concatenate([bq, bk, bv])[None, :].astype(np.float32))
        pwc = np.ascontiguousarray(
            proj_w[col:col + 128, :].reshape(2, HD, HIDDEN).astype(np.float32))
        in_maps.append({
            "x": xf, "w": wc, "brow": browc, "psgn": psgn, "pw": pwc,
            "cos_t": cos_t, "sin_t": sin_t,
        })
    return in_maps


_PROGRAM_CACHE = {}


def _get_program(NB, T):
    key = (NB, T)
    if key not in _PROGRAM_CACHE:
        _PROGRAM_CACHE[key] = build_program(NB, T)
    return _PROGRAM_CACHE[key]


def run(x, qkv_w, qkv_b, proj_w, proj_b, NB, T, trace=False):
    nc = _get_program(NB, T)
    in_maps = make_core_inputs(x, qkv_w, qkv_b, proj_w, NB, T)
    res = bass_utils.run_bass_kernel_spmd(
        nc, in_maps, core_ids=list(range(N_CORES)), trace=trace)
    acc = res.results[0]["yT"].astype(np.float32).copy()
    for c in range(1, N_CORES):
        acc += res.results[c]["yT"]
    out = (acc.T.reshape(NB, T, HIDDEN)
           + np.asarray(proj_b)[None, None, :].astype(np.float32))
    return out, res


def kernel(x, qkv_w, qkv_b, proj_w, proj_b):
    x = np.asarray(x)
    B, L, D = x.shape
    out, _ = run(x, np.asarray(qkv_w), np.asarray(qkv_b),
                 np.asarray(proj_w), np.asarray(proj_b), NB=B, T=L)
    return out.astype(np.float32)


# revision 16
# speedup vs baseline: 1.6197x; 1.0912x over previous
"""Trainium2 Bass kernel for nn_CausalSelfAttention (B=4, L=2048, D=1024, H=16).

Sharding: 2 heads per core (tensor parallel) x 8 cores, all batches on every
core.  Each core computes qkv for its 2 heads over all tokens (reading full x),
runs causal attention, and produces a partial projection output
yT_c = proj_w[rows_c].T @ O_c^T  of shape [D, B*L].  The host sums the 8
partials, transposes, and adds proj_b.

Device pipeline per core (all matmuls fp32r: 1 cycle/row at N>=256):
  x [tok,d] --PE transpose--> xT tiles [d,tok]
  qkvT [feat,tok] = w_tile.T @ xT (+bias via K=1 matmul)
  RoPE: rot(q) via signed-permutation matmul on PE, then
        qT_roped = qT*cos + rot(qT)*sin  (3 full-height DVE ops)
  V: PE-transpose back to tok-major, stored as [V|1] tiles
  S^T[k,q] = KT_slice.T @ QT_slice (heads interleaved -> PE row-group overlap)
  P^T = exp(S^T/8) (ACT, causal via affine_select on diagonal stripes)
  O^T[hd+1,q] += [V|1].T @ P^T
  normalize via reciprocal_approx_fast + gpsimd partition_broadcast + DVE mul
  yT += pw_h.T @ OT_h (two K=64 matmuls per tile)
The normalize+projection of iteration i-1 is emitted after the qkv phase of
iteration i so the PE never idles on the normalization chain (HAM stays warm).
"""

import numpy as np

import concourse.bass as bass  # noqa: F401
import concourse.tile as tile
from concourse import mybir, bacc
from concourse import bass_utils
from concourse.masks import make_identity

f32 = mybir.dt.float32
f32r = mybir.dt.float32r
AL = mybir.AluOpType
AF = mybir.ActivationFunctionType


class _Bacc(bacc.Bacc):
    """Bacc that pins all activations to the one table set holding both
    ln and exp (plus copy/identity), so the per-iteration Ln<->Exp pair
    doesn't thrash ACT_TABLE_LOADs (~2.7us each)."""

    def insert_act_table_loads(self):
        import bass_rust as _bass_rust
        from concourse.hw_specs import get_activation_tables

        has_activation = any(
            isinstance(i, mybir.InstActivation)
            for bb in self.main_func.blocks
            for i in bb.instructions
        )
        if not has_activation:
            return
        # act_func_set_id is positional: keep the full list order, but empty
        # every other set so the chooser can only pick the combined one.
        tables = [
            (k, v if k == "natural_log_exp_and_others" else set())
            for k, v in get_activation_tables(self.m.arch).items()
        ]
        _bass_rust.insert_act_table_loads(self, tables)

HIDDEN = 1024
HEADS = 16
HD = 64
ROPE_BASE = 10000.0
N_CORES = 8
H2 = 2           # heads per core
F = 3 * H2 * HD  # 384 qkv feature columns per core
QCH = 512        # token chunk = attention q granule
DT = HIDDEN // 128  # 8 d tiles


def build_program(NB, T):
    """Build the per-core Bass program: NB batches of T tokens each."""
    assert T % QCH == 0
    NTOK = NB * T
    NKT = T // 128  # k tiles per batch
    nc = _Bacc("TRN2", target_bir_lowering=False, debug=False,
               num_devices=N_CORES)

    x = nc.dram_tensor("x", [NTOK, HIDDEN], f32r, kind="ExternalInput").ap()
    w = nc.dram_tensor("w", [HIDDEN, F], f32r, kind="ExternalInput").ap()
    brow = nc.dram_tensor("brow", [1, F], f32r, kind="ExternalInput").ap()
    psgn = nc.dram_tensor("psgn", [128, 128], f32r, kind="ExternalInput").ap()
    pw = nc.dram_tensor("pw", [2, HD, HIDDEN], f32r, kind="ExternalInput").ap()
    cos_t = nc.dram_tensor("cos_t", [128, T], f32, kind="ExternalInput").ap()
    sin_t = nc.dram_tensor("sin_t", [128, T], f32, kind="ExternalInput").ap()
    yT = nc.dram_tensor("yT", [HIDDEN, NTOK], f32, kind="ExternalOutput").ap()

    with tile.TileContext(nc) as tc:
        with tc.tile_pool(name="const", bufs=1) as constp, \
             tc.tile_pool(name="resident", bufs=1) as resp, \
             tc.tile_pool(name="xload", bufs=6) as xp, \
             tc.tile_pool(name="xt", bufs=12) as xtp, \
             tc.tile_pool(name="rope", bufs=3) as ropep, \
             tc.tile_pool(name="qtcur", bufs=2) as qtp, \
             tc.tile_pool(name="pt", bufs=4) as ptp, \
             tc.tile_pool(name="ot", bufs=3) as otp, \
             tc.tile_pool(name="ysb", bufs=3) as yp, \
             tc.tile_pool(name="small", bufs=4) as smp, \
             tc.tile_pool(name="ps_s", bufs=2, space="PSUM") as ps_s_p, \
             tc.tile_pool(name="ps_o", bufs=4, space="PSUM") as ps_o_p, \
             tc.tile_pool(name="ps_m", bufs=2, space="PSUM") as ps_m_p:

            # ---- constants / residents ----
            ident_f = constp.tile([128, 128], f32)
            make_identity(nc, ident_f[:])
            ident = constp.tile([128, 128], f32r)
            nc.vector.tensor_copy(ident[:], ident_f[:])
            # w tiles: per d-tile, F columns
            w_sb = constp.tile([128, DT * F], f32r)
            for dt in range(DT):
                nc.sync.dma_start(w_sb[:, dt * F:(dt + 1) * F],
                                  w[dt * 128:(dt + 1) * 128, :])
            brow_sb = constp.tile([1, F], f32r)
            nc.sync.dma_start(brow_sb[:], brow[:])
            psgn_sb = constp.tile([128, 128], f32r)
            nc.sync.dma_start(psgn_sb[:], psgn[:])
            ones_f = constp.tile([128, 512], f32)
            nc.gpsimd.memset(ones_f[:], 1.0)
            ones_row = constp.tile([1, 512], f32r)
            nc.vector.tensor_copy(ones_row[:], ones_f[0:1, :])
            pw_sb = constp.tile([64, 2 * HIDDEN], f32r)
            for h in range(2):
                nc.sync.dma_start(pw_sb[:, h * HIDDEN:(h + 1) * HIDDEN], pw[h])
            cos_sb = constp.tile([128, T], f32)
            nc.sync.dma_start(cos_sb[:], cos_t[:])
            sin_sb = constp.tile([128, T], f32)
            nc.sync.dma_start(sin_sb[:], sin_t[:])

            KT_res = resp.tile([128, T], f32r)
            V_res = resp.tile([128, NKT * 130], f32r)
            v4 = V_res[:].rearrange("p (kt h c) -> p kt h c", kt=NKT, h=2)
            nc.gpsimd.tensor_copy(
                v4[:, :, :, 64],
                ones_f[:, :2 * NKT].rearrange("p (kt h) -> p kt h", kt=NKT))

            def norm_part(st):
                O, t0v = st
                ots = []
                for h in range(2):
                    # 1/rowsum = exp(-ln(rowsum)) on ACT (same table set as
                    # the attention Exp; DVE reciprocal is 3.3us and would
                    # stall the pipeline)
                    lnv = smp.tile([1, 512], f32, tag="ln", name="lnv")
                    nc.scalar.activation(lnv[:], O[h][64:65, :], AF.Ln)
                    rs_sb = smp.tile([1, 512], f32, tag="rs", name="rs")
                    nc.scalar.activation(rs_sb[:], lnv[:], AF.Exp,
                                         bias=0.0, scale=-1.0)
                    rsb = smp.tile([64, 512], f32, tag="rsb", name="rsb")
                    nc.gpsimd.partition_broadcast(rsb[:], rs_sb[:])
                    ot_t = otp.tile([64, 512], f32r, tag=f"ot{h}",
                                    name=f"ot{h}")
                    nc.vector.tensor_tensor(ot_t[:], O[h][0:64, :], rsb[:],
                                            AL.mult)
                    ots.append(ot_t)
                return ots

            def proj_part(st, ots):
                O, t0v = st
                for oi in range(8):
                    ps_y = ps_m_p.tile([128, 512], f32, tag="m", name="ps_y")
                    for h in range(2):
                        nc.tensor.matmul(
                            ps_y[:],
                            pw_sb[:, h * HIDDEN + oi * 128:
                                  h * HIDDEN + (oi + 1) * 128],
                            ots[h][:], start=(h == 0), stop=(h == 1))
                    ysb = yp.tile([128, 512], f32, tag="y", name="ysb")
                    if oi % 2 == 0:
                        nc.vector.tensor_copy(ysb[:], ps_y[:])
                    else:
                        nc.scalar.copy(ysb[:], ps_y[:])
                    nc.sync.dma_start(
                        yT[oi * 128:(oi + 1) * 128, t0v:t0v + 512], ysb[:])

            prev = None
            for b in range(NB):
                for qc in range(T // QCH):
                    Q0 = qc * QCH
                    t0 = b * T + Q0
                    prev_ots = norm_part(prev) if prev is not None else None
                    # ---------- qkv phase for tokens [t0, t0+512) ----------
                    xa = [xp.tile([128, HIDDEN], f32r, tag="x", name=f"xa{tt}")
                          for tt in range(4)]
                    for tt in range(4):
                        nc.sync.dma_start(
                            xa[tt][:], x[t0 + tt * 128: t0 + (tt + 1) * 128, :])
                    xt_sb = [xtp.tile([128, QCH], f32r, tag="xt",
                                      name=f"xt{dt}") for dt in range(DT)]
                    for dt in range(DT):
                        ps_xt = ps_m_p.tile([128, QCH], f32r, tag="m",
                                            name="ps_xt")
                        for tt in range(4):
                            nc.tensor.transpose(
                                ps_xt[:, tt * 128:(tt + 1) * 128],
                                xa[tt][:, dt * 128:(dt + 1) * 128], ident[:])
                        nc.vector.tensor_copy(xt_sb[dt][:], ps_xt[:])
                    QT_cur = qtp.tile([128, QCH], f32r, tag="qt", name="QT")
                    for f in range(3):  # 0=q, 1=k, 2=v
                        ps_f = ps_m_p.tile([128, QCH], f32, tag="m",
                                           name="ps_f")
                        for dt in range(DT):
                            nc.tensor.matmul(
                                ps_f[:],
                                w_sb[:, dt * F + f * 128:dt * F + (f + 1) * 128],
                                xt_sb[dt][:], start=(dt == 0), stop=False)
                        nc.tensor.matmul(
                            ps_f[:], brow_sb[:, f * 128:(f + 1) * 128],
                            ones_row[:], start=False, stop=True)
                        raw = ropep.tile([128, QCH], f32r, tag="raw",
                                         name="raw")
                        nc.scalar.copy(raw[:], ps_f[:])
                        if f < 2:
                            ps_rot = ps_m_p.tile([128, QCH], f32, tag="m",
                                                 name="ps_rot")
                            nc.tensor.matmul(ps_rot[:], psgn_sb[:], raw[:],
                                             start=True, stop=True)
                            t1 = ropep.tile([128, QCH], f32, tag="t1",
                                            name="t1")
                            nc.vector.tensor_tensor(
                                t1[:], raw[:], cos_sb[:, Q0:Q0 + QCH], AL.mult)
                            t2 = ropep.tile([128, QCH], f32, tag="t2",
                                            name="t2")
                            nc.vector.tensor_tensor(
                                t2[:], ps_rot[:], sin_sb[:, Q0:Q0 + QCH],
                                AL.mult)
                            dst = (QT_cur[:] if f == 0
                                   else KT_res[:, Q0:Q0 + QCH])
                            nc.vector.tensor_tensor(dst, t1[:], t2[:], AL.add)
                        else:
                            for tt in range(4):
                                ps_vt = ps_m_p.tile([128, 128], f32r, tag="m",
                                                    name="ps_vt")
                                nc.tensor.transpose(
                                    ps_vt[:],
                                    raw[:, tt * 128:(tt + 1) * 128], ident[:])
                                kt = Q0 // 128 + tt
                                nc.vector.tensor_copy(
                                    v4[:, kt, :, 0:64],
                                    ps_vt[:].rearrange("p (h j) -> p h j", h=2))
                    # ---------- deferred projection ----------
                    if prev is not None:
                        proj_part(prev, prev_ots)
                    # ---------- attention for (b, qc) ----------
                    nkb = (Q0 + QCH) // 128
                    O = [ps_o_p.tile([65, 512], f32, tag="o", name=f"O{h}")
                         for h in range(2)]
                    for kb in range(nkb):
                        qstart = max(0, 128 * kb - Q0)
                        for h in range(2):
                            hp = slice(64 * h, 64 * h + 64)
                            ps_sc = ps_s_p.tile([128, QCH], f32, tag="s",
                                                name="ps_sc")
                            nc.tensor.matmul(
                                ps_sc[:, qstart:QCH],
                                KT_res[hp, kb * 128:(kb + 1) * 128],
                                QT_cur[hp, qstart:QCH],
                                start=True, stop=True)
                            pt = ptp.tile([128, QCH], f32r, tag="pt",
                                          name="pt")
                            nc.scalar.activation(pt[:, qstart:QCH],
                                                 ps_sc[:, qstart:QCH],
                                                 AF.Exp, bias=0.0, scale=0.125)
                            if 128 * kb >= Q0:
                                ds = 128 * kb - Q0
                                nc.gpsimd.affine_select(
                                    out=pt[:, ds:ds + 128],
                                    in_=pt[:, ds:ds + 128],
                                    pattern=[[1, 128]], compare_op=AL.is_ge,
                                    fill=0.0, base=0, channel_multiplier=-1)
                            nc.tensor.matmul(
                                O[h][:, qstart:QCH],
                                V_res[:, 130 * kb + 65 * h:
                                      130 * kb + 65 * h + 65],
                                pt[:, qstart:QCH],
                                start=(kb == 0), stop=(kb == nkb - 1))
                    prev = (O, t0)
            proj_part(prev, norm_part(prev))
    nc.compile()
    return nc


# ---------------------------------------------------------------- host side

def _rope_tables(T):
    inv_freq = 1.0 / (ROPE_BASE ** (np.arange(0, HD, 2, dtype=np.float64) / HD))
    pos = np.arange(T, dtype=np.float64)
    ang = np.outer(pos, inv_freq)          # [T, 32]
    cos = np.cos(ang).astype(np.float32)   # [T, 32]
    sin = np.sin(ang).astype(np.float32)
    jm32 = np.arange(128) % 32
    # feat-major: row r (feature), col t (within-batch position)
    cos_t = np.ascontiguousarray(cos[:, jm32].T)   # [128, T]
    sin_t = np.ascontiguousarray(sin[:, jm32].T)
    return cos_t, sin_t


def _psgn():
    p = np.zeros((HD, HD), np.float32)
    for i in range(32):
        p[i + 32, i] = -1.0   # out dim i (<32) = -in dim i+32
        p[i, i + 32] = 1.0    # out dim i+32   = +in dim i
    pf = np.zeros((128, 128), np.float32)
    pf[0:64, 0:64] = p        # head A block
    pf[64:128, 64:128] = p    # head B block
    return np.ascontiguousarray(pf)


def make_core_inputs(x, qkv_w, qkv_b, proj_w, NB, T):
    NTOK = NB * T
    xf = np.ascontiguousarray(
        np.asarray(x).reshape(NTOK, HIDDEN).astype(np.float32))
    cos_t, sin_t = _rope_tables(T)
    psgn = _psgn()
    in_maps = []
    for c in range(N_CORES):
        col = HD * H2 * c
        wq = qkv_w[:, col:col + 128]
        wk = qkv_w[:, HIDDEN + col:HIDDEN + col + 128]
        wv = qkv_w[:, 2 * HIDDEN + col:2 * HIDDEN + col + 128]
        wc = np.ascontiguousarray(
            np.concatenate([wq, wk, wv], axis=1).astype(np.float32))
        bq = qkv_b[col:col + 128]
        bk = qkv_b[HIDDEN + col:HIDDEN + col + 128]
        bv = qkv_b[2 * HIDDEN + col:2 * HIDDEN + col + 128]
        browc = np.ascontiguousarray(
            np.concatenate([bq, bk, bv])[None, :].astype(np.float32))
        pwc = np.ascontiguousarray(
            proj_w[col:col + 128, :].reshape(2, HD, HIDDEN).astype(np.float32))
        in_maps.append({
            "x": xf, "w": wc, "brow": browc, "psgn": psgn, "pw": pwc,
            "cos_t": cos_t, "sin_t": sin_t,
        })
    return in_maps


_PROGRAM_CACHE = {}


def _get_program(NB, T):
    key = (NB, T)
    if key not in _PROGRAM_CACHE:
        _PROGRAM_CACHE[key] = build_program(NB, T)
    return _PROGRAM_CACHE[key]


def run(x, qkv_w, qkv_b, proj_w, proj_b, NB, T, trace=False):
    nc = _get_program(NB, T)
    in_maps = make_core_inputs(x, qkv_w, qkv_b, proj_w, NB, T)
    res = bass_utils.run_bass_kernel_spmd(
        nc, in_maps, core_ids=list(range(N_CORES)), trace=trace)
    acc = res.results[0]["yT"].astype(np.float32).copy()
    for c in range(1, N_CORES):
        acc += res.results[c]["yT"]
    out = (acc.T.reshape(NB, T, HIDDEN)
           + np.asarray(proj_b)[None, None, :].astype(np.float32))
    return out, res


def kernel(x, qkv_w, qkv_b, proj_w, proj_b):
    x = np.asarray(x)
    B, L, D = x.shape
    out, _ = run(x, np.asarray(qkv_w), np.asarray(qkv_b),
                 np.asarray(proj_w), np.asarray(proj_b), NB=B, T=L)
    return out.astype(np.float32)
